# revision 22
# baseline (speedup 1.0000x reference)
"""Bass/Tile TRN2 kernel for nn_FCMTLSTMFull (CNN + BiLSTM + attention + MoE head).

kernel(**inputs) takes FULL unsharded inputs and returns the FULL (8192, 1)
float32 output.  Batch sharded 8 ways (data parallel), params replicated, one
fused program SPMD on cores 0-7.

v2 design vs the 753us baseline:
 - conv2 runs TWO fp8 DoubleRow passes per (sample, mo) instead of three:
   pass0 pairs taps (t0_hi, t2_hi) at rhs pair-stride 2; pass1 pairs
   (t1_hi, t1_lo) at pair-stride 0 (full-precision center tap).  fp8
   ranges are re-centered (h1 x16, conv2 weights x8, folded into b2f/dwT)
   to cut subnormal quantization noise, which keeps overall rel-err at
   the 3-pass baseline level.
 - maxpool drain rebalanced: conv1 drains on ACT; conv2 PSUM first-touch
   is a DVE tensor_tensor max pairing (or ACT copy + DVE pair, knob C2R);
   the rest of the max pyramid runs span-wide on DVE in bf16 (2x mode);
   GPSIMD does the final bias+relu.
 - conv work is emitted piece-wise between LSTM gate groups so PE / ACT /
   DVE stay concurrently busy through the whole LSTM phase; PSUM is two
   shared 2-bank pools (double buffered) covering fp/conv1/gates/head and
   conv2/scores respectively.
"""

import math
import os
from contextlib import ExitStack

import numpy as np
import ml_dtypes

import concourse.bass as bass
import concourse.mybir as mybir
import concourse.tile as tile
from concourse import bacc
from concourse.bass import ds

F32 = mybir.dt.float32
BF16 = mybir.dt.bfloat16
FP8 = mybir.dt.float8e4
I32 = mybir.dt.int32
AF = mybir.ActivationFunctionType
ALU = mybir.AluOpType
AX = mybir.AxisListType
PM = mybir.MatmulPerfMode

BF = ml_dtypes.bfloat16
F8 = ml_dtypes.float8_e4m3fn

N_CORES = 8
B_FULL = 8192
IN = 184
H = 128
SEQ = 4
FPS = 46
EPS = 1e-5

LP = IN + 2          # padded per-sample length (186)
SPAN = 16            # samples per conv span
WSP = SPAN * LP      # span window elems (2976)

S_H = 16.0           # h1 fp8 scale (folded: c1w,b1f x16; praw scale below)
S_W = 8.0            # conv2 weight fp8 scale (w2 x8 -> praw = 128*y2)
S_Y = S_H * S_W      # praw scale; b2f x S_Y, dwT / S_Y

C2R = os.environ.get("C2R", "DDDDDDDD")   # conv2 drain route per tile (len 8)
C1R = os.environ.get("C1R", "AAAA")       # conv1 drain route per quad
TICKN = int(os.environ.get("TICKN", "3"))  # conv pieces per LSTM gate group
TICKA = int(os.environ.get("TICKA", "1"))  # conv pieces per attention group


def build_program(bc):
    nc = bacc.Bacc(None, debug=False)

    spans = bc // SPAN
    ncol = 512
    nh = bc // ncol

    di = {}

    def inp(name, shape, dt):
        di[name] = nc.dram_tensor(name, list(shape), dt, kind="ExternalInput")
        return di[name]

    # activations
    inp("xph", (1, bc * LP + 68), FP8)
    inp("xpl", (1, bc * LP + 68), FP8)
    inp("xTa", (128, bc), BF16)
    inp("xTb", (56, bc), BF16)
    inp("lab", (1, bc), BF16)
    inp("lab4", (4, bc), BF16)
    inp("onesd", (1, bc), BF16)
    # conv weights
    inp("c1w", (10, 2, 128), FP8)        # [k10, pair, m]
    inp("b1f", (128, 1), F32)
    inp("w2p", (128, 2, 2, 2, 128), FP8)  # [k, mo, pass, pair, m]
    inp("b2f", (128, 2), F32)
    # temporal (bf16)
    inp("fpTa", (128, IN), BF16)
    inp("fpTb", (56, IN), BF16)
    inp("fpb", (FPS, SEQ), F32)
    inp("wih0T", (47, 2, 512), BF16)     # row 46 = bias
    inp("whh0T", (128, 2, 512), BF16)
    inp("wih1T", (128, 2, 2, 512), BF16)
    inp("whh1T", (128, 2, 512), BF16)
    inp("b1l", (1, 2, 4, 128), BF16)     # lay1 bias rows [1, d, g, m]
    inp("awT", (128, 2, 2, 128), BF16)   # [k, ki, mo, m]
    inp("awb", (1, 2, 128), BF16)
    inp("av", (128, 2), BF16)
    # head (bf16)
    inp("dwT", (128, 2, 2, 128), BF16)   # [k, ki, mo, m]
    inp("e1T", (128, 4, 512), BF16)      # [k, ki, 4*128]
    inp("e1b", (1, 4, 128), BF16)
    inp("e2T", (128, 4, 256), BF16)
    inp("e2b", (1, 2, 128), BF16)
    inp("sw1T", (128, 2, 2, 128), BF16)  # [k, e, ki, m]
    inp("sb1", (1, 2, 128), BF16)
    inp("sw2T", (128, 2), BF16)
    inp("sb2", (1, 2), F32)
    inp("lw1T", (128, 2, 2, 2, 128), BF16)  # [k, e, ki, mo, m]
    inp("lb1", (1, 2, 2, 128), BF16)
    inp("lw2T", (128, 2, 2), BF16)          # [k, e, ki]
    inp("lb2", (1, 2), F32)
    inp("awbP", (128, 2), F32)
    inp("e1bP", (128, 4), F32)
    inp("e2bP", (128, 2), F32)
    inp("sb1P", (128, 2), F32)
    inp("lb1P", (128, 4), F32)
    inp("eb2P", (4, 1), F32)
    inp("iota4", (4, 1), F32)
    inp("ones4", (4, 1), F32)
    out_d = nc.dram_tensor("out", [bc, 1], F32, kind="ExternalOutput")

    with tile.TileContext(nc) as tc:
        with ExitStack() as ctx:
            wp = ctx.enter_context(tc.tile_pool(name="wp", bufs=1))
            pb = ctx.enter_context(tc.tile_pool(name="pb", bufs=1))
            wk = ctx.enter_context(tc.tile_pool(name="wk", bufs=2))
            cv = ctx.enter_context(tc.tile_pool(name="cv", bufs=2))
            yh = ctx.enter_context(tc.tile_pool(name="yh", bufs=2))
            # PSUM: two shared 2-bank pools, double-buffered (8 banks total)
            psA = ctx.enter_context(tc.tile_pool(name="psA", bufs=2,
                                                 space="PSUM"))
            psB = ctx.enter_context(tc.tile_pool(name="psB", bufs=2,
                                                 space="PSUM"))

            W = {}
            _early = ["c1w", "b1f", "w2p", "b2f", "fpTa", "fpTb", "fpb",
                      "wih0T", "whh0T", "xTa", "xTb"]
            _late = []

            def _load(name):
                d = di[name]
                t = wp.tile(list(d.shape), d.dtype, name=f"W{name}",
                            tag=f"W{name}")
                nc.sync.dma_start(t[:], d[:])
                W[name] = t

            for name in di:
                if name in ("xph", "xpl", "lab", "lab4", "onesd"):
                    continue
                if name in _early:
                    _load(name)
                else:
                    _late.append(name)
            xTlo = W["xTa"][:]
            xThi = W["xTb"][:]

            ones = pb.tile([1, bc], BF16, name="ones", tag="ones")
            nc.sync.dma_start(ones[:], di["onesd"][:])

            # ------------- conv state -------------
            pool_ = pb.tile([128, 2, bc], BF16, name="pool", tag="pool")

            h1sp = ctx.enter_context(tc.tile_pool(name="h1sp", bufs=2))
            h1s_bufs = [h1sp.tile([128, WSP + 8], FP8, name=f"h1s{i}",
                                  tag="h1s") for i in range(2)]
            for b in h1s_bufs:
                nc.gpsimd.memset(b[:], 0.0)
            NA = C2R.count("A")   # A-route tiles per span (pattern len 8)
            assert len(C2R) == 8 and (NA == 0 or 8 % NA == 0)
            ysp_bufs = [yh.tile([128, max(NA, 1) * 2, 2, IN], BF16,
                                name=f"yspA{i}", tag="yspA") for i in range(2)]
            praw_bufs = [wk.tile([128, SPAN, 2], BF16, tag="praw", bufs=2,
                                 name=f"praw{i}") for i in range(2)]

            xp10_cur = [None]
            phase_route = ["P"]   # P = use C2R pattern; A/D = forced

            def emit_conv1(sp, qi):
                if qi == 0:
                    xp10 = cv.tile([10, WSP + 12], FP8, tag="xp10",
                                   name=f"xp10_{sp}")
                    s0 = sp * SPAN
                    nc.sync.dma_start(
                        xp10[0:5, :],
                        bass.AP(di["xph"], s0 * LP, [[1, 5], [1, WSP + 12]]))
                    nc.sync.dma_start(
                        xp10[5:10, :],
                        bass.AP(di["xpl"], s0 * LP, [[1, 5], [1, WSP + 12]]))
                    xp10_cur[0] = xp10
                xv = xp10_cur[0][:]
                h1v = h1s_bufs[sp % 2][:]
                pc1 = psA.tile([128, 4, 256], F32, tag="pa",
                               name=f"c1_{sp}_{qi}")
                for si in range(4):
                    s = qi * 4 + si
                    src = bass.AP(xv.tensor, xv.offset + s * LP,
                                  [xv.ap[0]] + [[2, 2], [1, IN]])
                    nc.tensor.matmul(pc1[:, si, 0:IN], W["c1w"][:], src,
                                     start=True, stop=True,
                                     perf_mode=PM.DoubleRow)
                dst = bass.AP(h1v.tensor, h1v.offset + (qi * 4) * LP + 1,
                              [h1v.ap[0]] + [[LP, 4], [1, IN]])
                route = C1R[qi % len(C1R)]
                if route == "A":
                    nc.scalar.activation(dst, pc1[:, :, 0:IN], AF.Relu,
                                         bias=W["b1f"][:, 0:1], scale=1.0)
                else:
                    nc.vector.tensor_scalar(dst, pc1[:, :, 0:IN],
                                            W["b1f"][:, 0:1], 0.0,
                                            ALU.add, ALU.max)

            def emit_conv2(sp, qi):
                h1v = h1s_bufs[sp % 2][:]
                yspA = ysp_bufs[sp % 2]
                praw_s = praw_bufs[sp % 2]
                for ti in range(2):
                    tidx = qi * 2 + ti
                    p2 = psB.tile([128, 2, 2, 256], F32, tag="pb",
                                  name=f"c2_{sp}_{tidx}")
                    for si in range(2):
                        s = tidx * 2 + si
                        base = s * LP
                        srcA = bass.AP(h1v.tensor, h1v.offset + base,
                                       [h1v.ap[0]] + [[2, 2], [1, IN]])
                        srcB = bass.AP(h1v.tensor, h1v.offset + base + 1,
                                       [h1v.ap[0]] + [[0, 2], [1, IN]])
                        for mo in range(2):
                            nc.tensor.matmul(p2[:, si, mo, 0:IN],
                                             W["w2p"][:, mo, 0], srcA,
                                             start=True, stop=False,
                                             perf_mode=PM.DoubleRow)
                            nc.tensor.matmul(p2[:, si, mo, 0:IN],
                                             W["w2p"][:, mo, 1], srcB,
                                             start=False, stop=True,
                                             perf_mode=PM.DoubleRow)
                    route = C2R[tidx % len(C2R)]
                    if route == "A":
                        aslot = C2R[:tidx].count("A")
                        nc.scalar.activation(
                            yspA[:, ds(aslot * 2, 2), :, :],
                            p2[:, :, :, 0:IN], AF.Copy)
                    else:
                        nc.vector.tensor_reduce(
                            praw_s[:, ds(tidx * 2, 2), :],
                            p2[:, :, :, 0:IN], axis=AX.X, op=ALU.max)

            def emit_chain(sp):
                praw_s = praw_bufs[sp % 2]
                if NA > 0:
                    yspA = ysp_bufs[sp % 2]
                    na2 = NA * 2
                    t1 = wk.tile([128, na2, 2, 92], BF16, tag="pyr1", bufs=2,
                                 name=f"t1_{sp}")
                    nc.vector.tensor_tensor(t1[:], yspA[:, :, :, 0:92],
                                            yspA[:, :, :, 92:184], ALU.max)
                    t2 = wk.tile([128, na2, 2, 46], BF16, tag="pyr2", bufs=2,
                                 name=f"t2_{sp}")
                    nc.vector.tensor_tensor(t2[:], t1[:, :, :, 0:46],
                                            t1[:, :, :, 46:92], ALU.max)
                    t3 = wk.tile([128, na2, 2, 23], BF16, tag="pyr3", bufs=2,
                                 name=f"t3_{sp}")
                    nc.vector.tensor_tensor(t3[:], t2[:, :, :, 0:23],
                                            t2[:, :, :, 23:46], ALU.max)
                    # scatter the A-tile results into praw (stride 8/NA tiles)
                    pv = praw_s[:]
                    adst = bass.AP(pv.tensor, pv.offset,
                                   [pv.ap[0]] + [[2 * 16 // NA, NA],
                                                 [2, 2], [1, 2]])
                    nc.vector.tensor_reduce(adst, t3[:], axis=AX.X,
                                            op=ALU.max)
                # bias + relu on gpsimd -> pool
                for mo in range(2):
                    nc.gpsimd.tensor_scalar(
                        pool_[:, mo, ds(sp * SPAN, SPAN)], praw_s[:, :, mo],
                        W["b2f"][:, mo:mo + 1], 0.0, ALU.add, ALU.max)

            TOTAL_PIECES = (spans + 1) * 4
            piece_ctr = [0]

            def tick(n=1):
                for _ in range(n):
                    p = piece_ctr[0]
                    if p >= TOTAL_PIECES:
                        return
                    piece_ctr[0] += 1
                    sp, qi = p // 4, p % 4
                    if sp < spans:
                        emit_conv1(sp, qi)
                    if sp >= 1:
                        emit_conv2(sp - 1, qi)
                        if qi == 3:
                            emit_chain(sp - 1)

            # ---------------- fp projection -> xt tiles (47 rows) ---------
            tick(int(os.environ.get("TICK0", "6")))
            xt = [pb.tile([47, bc], BF16, name=f"xt{t}", tag=f"xt{t}")
                  for t in range(SEQ)]
            for t in range(SEQ):
                nc.sync.dma_start(xt[t][46:47, :], di["onesd"][:])
            for t in range(SEQ):
                for n in range(nh):
                    cols = ds(n * ncol, ncol)
                    ps = psA.tile([46, 1024], F32, tag="pa", name=f"fp{t}_{n}")
                    nc.tensor.matmul(ps[:, 0:ncol],
                                     W["fpTa"][:, ds(t * FPS, FPS)],
                                     xTlo[:, cols], start=True, stop=False)
                    nc.tensor.matmul(ps[:, 0:ncol],
                                     W["fpTb"][:, ds(t * FPS, FPS)],
                                     xThi[:, cols], start=False, stop=True)
                    nc.scalar.activation(xt[t][0:46, cols], ps[:, 0:ncol],
                                         AF.Identity,
                                         bias=W["fpb"][:, t:t + 1], scale=1.0)
                    tick(1)

            for name in _late:
                _load(name)

            # ---------------- LSTM ----------------
            h0 = {}
            h1 = {}
            for (lay, hs) in ((0, h0), (1, h1)):
                for t in range(SEQ):
                    for d in range(2):
                        hs[(t, d)] = pb.tile([128, bc], BF16,
                                             name=f"h{lay}_{t}_{d}",
                                             tag=f"h{lay}_{t}_{d}")

            def lstm_dir(lay, hs, d):
                h_prev = None
                c_prev = None
                for step in range(SEQ):
                    t = step if d == 0 else SEQ - 1 - step
                    c_cur = wk.tile([128, bc], BF16, tag="c", bufs=2,
                                    name=f"c{lay}_{d}_{step}")
                    gv = {}
                    for gpair in ((0, 1), (2, 3)):
                        gt = wk.tile([128, 2, bc], BF16, tag=f"gt{gpair[0]}",
                                     bufs=2, name=f"gt{lay}_{d}_{step}_{gpair[0]}")
                        for n in range(nh):
                            cols = ds(n * ncol, ncol)
                            ps = psA.tile([128, 2, 512], F32, tag="pa",
                                          name=f"g{lay}_{d}_{step}_{gpair[0]}_{n}")
                            for gi_, g in enumerate(gpair):
                                if step == 0 and g == 1:
                                    # f-gate unused at step 0; cheap filler
                                    nc.tensor.matmul(
                                        ps[:, gi_, 0:ncol], W["b1l"][:, d, g],
                                        ones[0:1, cols], start=True, stop=True)
                                    continue
                                gs = ds(g * 128, 128)
                                if lay == 0:
                                    nc.tensor.matmul(
                                        ps[:, gi_, 0:ncol],
                                        W["wih0T"][:, d, gs], xt[t][:, cols],
                                        start=True, stop=(step == 0))
                                else:
                                    nc.tensor.matmul(
                                        ps[:, gi_, 0:ncol],
                                        W["wih1T"][:, d, 0, gs],
                                        h0[(t, 0)][:, cols],
                                        start=True, stop=False)
                                    nc.tensor.matmul(
                                        ps[:, gi_, 0:ncol],
                                        W["wih1T"][:, d, 1, gs],
                                        h0[(t, 1)][:, cols],
                                        start=False, stop=False)
                                    nc.tensor.matmul(
                                        ps[:, gi_, 0:ncol],
                                        W["b1l"][:, d, g], ones[0:1, cols],
                                        start=False, stop=(step == 0))
                                if step > 0:
                                    nc.tensor.matmul(
                                        ps[:, gi_, 0:ncol],
                                        W[f"whh{lay}T"][:, d, gs],
                                        h_prev[:, cols],
                                        start=False, stop=True)
                            if gpair == (0, 1):
                                nc.scalar.activation(gt[:, :, cols],
                                                     ps[:, :, 0:ncol],
                                                     AF.Sigmoid)
                            else:
                                nc.scalar.activation(gt[:, 0, cols],
                                                     ps[:, 0, 0:ncol],
                                                     AF.Tanh)
                                nc.scalar.activation(gt[:, 1, cols],
                                                     ps[:, 1, 0:ncol],
                                                     AF.Sigmoid)
                            tick(TICKN)
                        gv[gpair] = gt
                    g_if = gv[(0, 1)]
                    g_go = gv[(2, 3)]
                    if step == 0:
                        nc.vector.tensor_mul(c_cur[:], g_if[:, 0, :],
                                             g_go[:, 0, :])
                    else:
                        ig = wk.tile([128, bc], BF16, tag="ig", bufs=1,
                                     name=f"ig{lay}_{d}_{step}")
                        nc.vector.tensor_mul(ig[:], g_if[:, 0, :], g_go[:, 0, :])
                        nc.vector.tensor_mul(c_cur[:], g_if[:, 1, :], c_prev[:])
                        nc.vector.tensor_add(c_cur[:], c_cur[:], ig[:])
                    tch = wk.tile([128, bc], BF16, tag="tch", bufs=1,
                                  name=f"tc{lay}_{d}_{step}")
                    nc.scalar.activation(tch[:], c_cur[:], AF.Tanh)
                    nc.vector.tensor_mul(hs[(t, d)][:], g_go[:, 1, :], tch[:])
                    h_prev = hs[(t, d)]
                    c_prev = c_cur

            lstm_dir(0, h0, 0)
            lstm_dir(0, h0, 1)
            lstm_dir(1, h1, 0)
            lstm_dir(1, h1, 1)

            # ---------------- attention ----------------
            E = [pb.tile([1, bc], BF16, name=f"E{t}", tag=f"E{t}")
                 for t in range(SEQ)]
            for t in range(SEQ):
                u = wk.tile([128, 2, bc], BF16, tag="u", bufs=1, name=f"u{t}")
                for n in range(nh):
                    cols = ds(n * ncol, ncol)
                    ps = psA.tile([128, 2, 512], F32, tag="pa",
                                  name=f"at{t}_{n}")
                    for mo in range(2):
                        nc.tensor.matmul(ps[:, mo, 0:ncol],
                                         W["awT"][:, 0, mo], h1[(t, 0)][:, cols],
                                         start=True, stop=False)
                        nc.tensor.matmul(ps[:, mo, 0:ncol],
                                         W["awT"][:, 1, mo], h1[(t, 1)][:, cols],
                                         start=False, stop=False)
                        nc.tensor.matmul(ps[:, mo, 0:ncol],
                                         W["awb"][:, mo], ones[0:1, cols],
                                         start=False, stop=True)
                    nc.scalar.activation(u[:, :, cols], ps[:, :, 0:ncol],
                                         AF.Tanh)
                    tick(TICKA)
                for n in range(nh):
                    cols = ds(n * ncol, ncol)
                    ps = psB.tile([1, 1024], F32, tag="pb", name=f"sc{t}_{n}")
                    nc.tensor.matmul(ps[0:1, 0:ncol], W["av"][:, 0:1],
                                     u[:, 0, cols], start=True, stop=False)
                    nc.tensor.matmul(ps[0:1, 0:ncol], W["av"][:, 1:2],
                                     u[:, 1, cols], start=False, stop=True)
                    nc.scalar.activation(E[t][0:1, cols], ps[0:1, 0:ncol],
                                         AF.Exp)
                    tick(TICKA)
            SE = wk.tile([1, bc], BF16, tag="se", bufs=1, name="SE")
            nc.vector.tensor_add(SE[:], E[0][:], E[1][:])
            nc.vector.tensor_add(SE[:], SE[:], E[2][:])
            nc.vector.tensor_add(SE[:], SE[:], E[3][:])
            Rr = pb.tile([1, bc], BF16, name="Rr", tag="Rr")
            with nc.allow_low_precision("softmax weights tolerate bf16"):
                nc.vector.reciprocal(Rr[:], SE[:])
            ctxb = pb.tile([128, 2, bc], BF16, name="ctxb", tag="ctxb")
            for s in range(SEQ):
                As = wk.tile([1, bc], BF16, tag="As", bufs=2, name=f"As{s}")
                nc.vector.tensor_mul(As[:], E[s][:], Rr[:])
                AW = wk.tile([128, bc], BF16, tag="AW", bufs=1, name=f"AW{s}")
                nc.gpsimd.partition_broadcast(AW[:], As[0:1, :], channels=128)
                for p in range(2):
                    if s == 0:
                        nc.vector.tensor_mul(ctxb[:, p, :], h1[(0, p)][:], AW[:])
                    else:
                        cm = wk.tile([128, bc], BF16, tag="cm", bufs=1,
                                     name=f"cm{s}_{p}")
                        nc.vector.tensor_mul(cm[:], h1[(s, p)][:], AW[:])
                        nc.vector.tensor_add(ctxb[:, p, :], ctxb[:, p, :],
                                             cm[:])
                tick(1)

            # ---------------- head ----------------
            tick(TOTAL_PIECES)   # flush any remaining conv pieces
            zsp = pb.tile([128, 2, bc], BF16, name="zsp", tag="zsp")
            for n in range(nh):
                cols = ds(n * ncol, ncol)
                ps = psA.tile([128, 2, 512], F32, tag="pa", name=f"sp_{n}")
                for mo in range(2):
                    for ki in range(2):
                        nc.tensor.matmul(ps[:, mo, 0:ncol],
                                         W["dwT"][:, ki, mo],
                                         pool_[:, ki, cols],
                                         start=(ki == 0), stop=(ki == 1))
                nc.scalar.activation(zsp[:, :, cols], ps[:, :, 0:ncol], AF.Copy)
                tick(1)
            z2 = pb.tile([128, 4, bc], BF16, name="z2", tag="z2")
            for mop in range(2):
                for n in range(nh):
                    cols = ds(n * ncol, ncol)
                    ps = psB.tile([128, 2, 512], F32, tag="pb",
                                  name=f"e1_{mop}_{n}")
                    for mi in range(2):
                        mo = mop * 2 + mi
                        ms = ds(mo * 128, 128)
                        nc.tensor.matmul(ps[:, mi, 0:ncol], W["e1T"][:, 0, ms],
                                         zsp[:, 0, cols], start=True, stop=False)
                        nc.tensor.matmul(ps[:, mi, 0:ncol], W["e1T"][:, 1, ms],
                                         zsp[:, 1, cols], start=False, stop=False)
                        nc.tensor.matmul(ps[:, mi, 0:ncol], W["e1T"][:, 2, ms],
                                         ctxb[:, 0, cols], start=False, stop=False)
                        nc.tensor.matmul(ps[:, mi, 0:ncol], W["e1T"][:, 3, ms],
                                         ctxb[:, 1, cols], start=False, stop=False)
                        nc.tensor.matmul(ps[:, mi, 0:ncol], W["e1b"][:, mo],
                                         ones[0:1, cols], start=False, stop=True)
                    nc.scalar.activation(z2[:, ds(mop * 2, 2), cols],
                                         ps[:, :, 0:ncol], AF.Relu)
                    tick(1)
            enc = pb.tile([128, 2, bc], BF16, name="enc", tag="enc")
            for n in range(nh):
                cols = ds(n * ncol, ncol)
                ps = psB.tile([128, 2, 512], F32, tag="pb", name=f"e2_{n}")
                for mo in range(2):
                    ms = ds(mo * 128, 128)
                    for ki in range(4):
                        nc.tensor.matmul(ps[:, mo, 0:ncol], W["e2T"][:, ki, ms],
                                         z2[:, ki, cols],
                                         start=(ki == 0), stop=False)
                    nc.tensor.matmul(ps[:, mo, 0:ncol], W["e2b"][:, mo],
                                     ones[0:1, cols], start=False, stop=True)
                nc.scalar.activation(enc[:, :, cols], ps[:, :, 0:ncol], AF.Relu)
                tick(1)

            # experts + routed select
            pred = pb.tile([1, bc], F32, name="pred", tag="pred")
            Lf = pb.tile([1, bc], BF16, name="Lf", tag="Lf")
            nc.sync.dma_start(Lf[:], di["lab"][:])
            pred = pb.tile([1, bc], F32, name="pred", tag="pred")
            for n in range(nh):
                cols = ds(n * ncol, ncol)
                for e in range(4):
                    po = psB.tile([1, 1024], F32, tag="pb", name=f"xo{e}_{n}")
                    if e < 2:
                        ph = psA.tile([128, 2, 512], F32, tag="pa",
                                      name=f"xh{e}_{n}")
                        for ki in range(2):
                            nc.tensor.matmul(ph[:, 0, 0:ncol],
                                             W["sw1T"][:, e, ki],
                                             enc[:, ki, cols],
                                             start=(ki == 0), stop=(ki == 1))
                        hh = wk.tile([128, ncol], BF16, tag="hh", bufs=2,
                                     name=f"hhS{e}_{n}")
                        nc.scalar.activation(hh[:], ph[:, 0, 0:ncol], AF.Relu,
                                             bias=W["sb1P"][:, e:e + 1],
                                             scale=1.0)
                        nc.tensor.matmul(po[0:1, 0:ncol], W["sw2T"][:, e:e + 1],
                                         hh[:], start=True, stop=True)
                        b2ap = W["sb2"][0:1, e:e + 1]
                    else:
                        el = e - 2
                        ph = psA.tile([128, 2, 512], F32, tag="pa",
                                      name=f"xhL{el}_{n}")
                        for mo in range(2):
                            for ki in range(2):
                                nc.tensor.matmul(ph[:, mo, 0:ncol],
                                                 W["lw1T"][:, el, ki, mo],
                                                 enc[:, ki, cols],
                                                 start=(ki == 0), stop=(ki == 1))
                        hh = wk.tile([128, 2, ncol], BF16, tag="hhL", bufs=2,
                                     name=f"hhL{el}_{n}")
                        for mo in range(2):
                            nc.scalar.activation(hh[:, mo, :], ph[:, mo, 0:ncol],
                                                 AF.Relu,
                                                 bias=W["lb1P"][:, 2 * el + mo:
                                                                2 * el + mo + 1],
                                                 scale=1.0)
                        for mo in range(2):
                            nc.tensor.matmul(po[0:1, 0:ncol],
                                             W["lw2T"][:, el, mo:mo + 1],
                                             hh[:, mo, :],
                                             start=(mo == 0), stop=(mo == 1))
                        b2ap = W["lb2"][0:1, el:el + 1]
                    oe = wk.tile([1, ncol], F32, tag="oe", bufs=2,
                                 name=f"oe{e}_{n}")
                    nc.scalar.activation(oe[:], po[0:1, 0:ncol], AF.Identity,
                                         bias=b2ap, scale=1.0)
                    mk = wk.tile([1, ncol], BF16, tag="mk", bufs=2,
                                 name=f"mk{e}_{n}")
                    nc.vector.tensor_scalar(mk[:], Lf[0:1, cols], float(e),
                                            None, ALU.is_equal)
                    if e == 0:
                        nc.vector.tensor_mul(pred[0:1, cols], oe[:], mk[:])
                    else:
                        pm = wk.tile([1, ncol], F32, tag="pm", bufs=2,
                                     name=f"pm{e}_{n}")
                        nc.vector.tensor_mul(pm[:], oe[:], mk[:])
                        nc.vector.tensor_add(pred[0:1, cols], pred[0:1, cols],
                                             pm[:])
                tick(2)
            nc.sync.dma_start(out_d[:].rearrange("b one -> one b"), pred[:])

    nc.compile()
    return nc


_PROG_CACHE = {}


def _get_program(bc):
    if bc not in _PROG_CACHE:
        _PROG_CACHE[bc] = build_program(bc)
    return _PROG_CACHE[bc]


def prep_arrays(inputs, n_cores=N_CORES):
    f32 = np.float32
    gi = {k: np.asarray(v) for k, v in inputs.items()}
    x = gi["x"].astype(f32)
    lab = gi["group_labels"].astype(np.int32)
    B = x.shape[0]
    bc = B // n_cores

    s = f32(1.0 / math.sqrt(1.0 + EPS))
    g1 = gi["bn1_g"] * s
    b1f = (gi["conv1_b"] + gi["bn1_b"] / g1) * S_H
    w2 = gi["conv2_w"] * g1[None, :, None] * S_W   # (256,128,3), x8
    g2 = gi["bn2_g"] * s
    b2f = ((gi["conv2_b"] + gi["bn2_b"] / g2) * S_Y).reshape(2, 128).T
    dw = gi["cnn_dw"] * g2[None, :] / S_Y          # (256,256)

    # conv1 weights (x S_H): hi/lo residual split, 10-row layout
    # rows 0-4 = x_hi shifts 0..4, rows 5-9 = x_lo shifts 0..4
    w1 = gi["conv1_w"][:, 0, :].T.astype(f32) * S_H   # (3, 128) [tap, c]
    w1hi = w1.astype(F8).astype(f32)
    w1lo = (w1 - w1hi).astype(F8).astype(f32)
    c1w = np.zeros((10, 2, 128), f32)
    c1w[2:5, 0] = w1lo       # W_lo . X_hi
    c1w[7:10, 0] = w1hi      # W_hi . X_lo
    c1w[0:3, 1] = w1hi       # W_hi . X_hi
    c1w[5:8, 1] = w1lo       # W_lo . X_lo
    c1w = c1w.astype(F8)

    # conv2 DR weights: [k, mo, pass, pair, m]
    # pass0 pair (t0_hi, t2_hi) @ stride 2; pass1 pair (t1_hi, t1_lo) @ str 0
    w2p = np.zeros((128, 2, 2, 2, 128), f32)
    wr = w2.reshape(2, 128, 128, 3)           # [mo, m, k, t]
    for mo in range(2):
        t0 = wr[mo, :, :, 0].T
        t1 = wr[mo, :, :, 1].T
        t2 = wr[mo, :, :, 2].T
        t0h = t0.astype(F8).astype(f32)
        t2h = t2.astype(F8).astype(f32)
        t1h = t1.astype(F8).astype(f32)
        w2p[:, mo, 0, 0] = t0h
        w2p[:, mo, 0, 1] = t2h
        w2p[:, mo, 1, 0] = t1h
        w2p[:, mo, 1, 1] = t1 - t1h
    w2p = w2p.astype(F8)

    fpT = gi["fp_w"].T.astype(f32)
    fpb = np.ascontiguousarray(gi["fp_b"].reshape(SEQ, FPS).T).astype(f32)

    def pack_T(w):
        return np.ascontiguousarray(w.T)

    b0 = np.stack([gi["bih0"][d] + gi["bhh0"][d] for d in range(2)])  # (2,512)
    wih0T = np.zeros((47, 2, 512), f32)
    for d in range(2):
        wih0T[0:46, d] = pack_T(gi["Wih0"][d])
        wih0T[46, d] = b0[d]
    whh0T = np.stack([pack_T(gi["Whh0"][d]) for d in range(2)], axis=1)
    wih1 = np.stack([pack_T(gi["Wih1"][d]) for d in range(2)], axis=1)  # (256,2,512)
    wih1T = np.ascontiguousarray(
        wih1.reshape(2, 128, 2, 512).transpose(1, 2, 0, 3))
    whh1T = np.stack([pack_T(gi["Whh1"][d]) for d in range(2)], axis=1)
    b1l = np.stack([gi["bih1"][d] + gi["bhh1"][d] for d in range(2)])
    b1l = np.ascontiguousarray(b1l.reshape(1, 2, 4, 128))

    awT = np.ascontiguousarray(
        gi["attW_w"].T.reshape(2, 128, 2, 128).transpose(1, 0, 2, 3))
    awb = np.ascontiguousarray(gi["attW_b"].reshape(1, 2, 128))
    av = np.ascontiguousarray(gi["attv"].reshape(2, 128).T)

    dwT = np.ascontiguousarray(
        dw.reshape(2, 128, 2, 128).transpose(3, 2, 0, 1))

    e1b_fold = gi["enc1_b"] + gi["enc1_w"][:, :256] @ gi["cnn_db"]
    es1 = np.ones(512, f32) * s * gi["bne1_g"]
    e1w = gi["enc1_w"] * es1[:, None]
    e1b = (e1b_fold * es1 + gi["bne1_b"]).reshape(1, 4, 128)
    e1T = np.ascontiguousarray(
        e1w.T.reshape(4, 128, 512).transpose(1, 0, 2))
    es2 = np.ones(256, f32) * s * gi["bne2_g"]
    e2w = gi["enc2_w"] * es2[:, None]
    e2b = (gi["enc2_b"] * es2 + gi["bne2_b"]).reshape(1, 2, 128)
    e2T = np.ascontiguousarray(
        e2w.T.reshape(4, 128, 256).transpose(1, 0, 2))

    sw1T = np.ascontiguousarray(
        np.stack([gi["decS_w1"][e].T.reshape(2, 128, 128) for e in range(2)],
                 axis=0).transpose(2, 0, 1, 3))
    sb1 = np.ascontiguousarray(
        np.stack([gi["decS_b1"][e] for e in range(2)], axis=0).reshape(1, 2, 128))
    sw2T = np.ascontiguousarray(
        np.stack([gi["decS_w2"][e, 0] for e in range(2)], axis=1))
    sb2 = gi["decS_b2"].reshape(1, 2).astype(f32)
    lw1T = np.ascontiguousarray(
        np.stack([gi["decL_w1"][e].T.reshape(2, 128, 2, 128) for e in range(2)],
                 axis=0).transpose(2, 0, 1, 3, 4))
    lb1 = np.ascontiguousarray(
        np.stack([gi["decL_b1"][e].reshape(2, 128) for e in range(2)],
                 axis=0).reshape(1, 2, 2, 128))
    lw2T = np.ascontiguousarray(
        np.stack([gi["decL_w2"][e, 0].reshape(2, 128) for e in range(2)],
                 axis=0).transpose(2, 0, 1))
    lb2 = gi["decL_b2"].reshape(1, 2).astype(f32)

    awbP = np.ascontiguousarray(awb[0].T).astype(f32)          # (128,2)
    e1bP = np.ascontiguousarray(e1b[0].T).astype(f32)          # (128,4)
    e2bP = np.ascontiguousarray(e2b[0].T).astype(f32)          # (128,2)
    sb1P = np.ascontiguousarray(sb1[0].T).astype(f32)          # (128,2)
    lb1P = np.ascontiguousarray(
        lb1[0].reshape(4, 128).T).astype(f32)                  # (128,4)
    eb2P = np.array([[sb2[0, 0]], [sb2[0, 1]],
                     [lb2[0, 0]], [lb2[0, 1]]], f32)           # (4,1)
    iota4 = np.arange(4, dtype=f32).reshape(4, 1)
    ones4 = np.ones((4, 1), f32)

    shared = dict(
        awbP=awbP, e1bP=e1bP, e2bP=e2bP, sb1P=sb1P, lb1P=lb1P,
        eb2P=eb2P, iota4=iota4, ones4=ones4,
        c1w=c1w, b1f=b1f.reshape(128, 1).astype(f32),
        w2p=w2p, b2f=np.ascontiguousarray(b2f).astype(f32),
        fpTa=np.ascontiguousarray(fpT[0:128]).astype(BF),
        fpTb=np.ascontiguousarray(fpT[128:IN]).astype(BF),
        fpb=fpb,
        wih0T=wih0T.astype(BF), whh0T=whh0T.astype(BF),
        wih1T=wih1T.astype(BF), whh1T=whh1T.astype(BF),
        b1l=b1l.astype(BF),
        awT=awT.astype(BF), awb=awb.astype(BF), av=av.astype(BF),
        dwT=dwT.astype(BF),
        e1T=e1T.astype(BF), e1b=e1b.astype(BF),
        e2T=e2T.astype(BF), e2b=e2b.astype(BF),
        sw1T=sw1T.astype(BF), sb1=sb1.astype(BF), sw2T=sw2T.astype(BF),
        sb2=sb2,
        lw1T=lw1T.astype(BF), lb1=lb1.astype(BF), lw2T=lw2T.astype(BF),
        lb2=lb2,
    )
    per_core = []
    xhi = x.astype(F8).astype(f32)
    xlo = (x - xhi).astype(F8).astype(f32)
    for c in range(n_cores):
        sl = slice(c * bc, (c + 1) * bc)

        def flat_pad(xa):
            flat = np.zeros((1, bc * LP + 68), f32)
            flat[0, 2:2 + bc * LP] = np.pad(xa[sl], ((0, 0), (1, 1))).reshape(-1)
            return flat.astype(F8)

        per_core.append(dict(
            xph=flat_pad(xhi),
            xpl=flat_pad(xlo),
            xTa=np.ascontiguousarray(x[sl].T[0:128]).astype(BF),
            xTb=np.ascontiguousarray(x[sl].T[128:IN]).astype(BF),
            onesd=np.ones((1, bc), BF),
            lab=np.ascontiguousarray(lab[sl].reshape(1, bc).astype(BF)),
            lab4=np.ascontiguousarray(
                np.broadcast_to(lab[sl].reshape(1, bc), (4, bc)).astype(BF)),
        ))
    return shared, per_core, bc


def kernel(**inputs) -> np.ndarray:
    from concourse.bass_utils import run_bass_kernel_spmd

    shared, per_core, bc = prep_arrays(inputs)
    nc = _get_program(bc)
    in_maps = [dict(shared, **pc) for pc in per_core]
    res = run_bass_kernel_spmd(nc, in_maps, core_ids=list(range(N_CORES)))
    out = np.concatenate([res.results[c]["out"] for c in range(N_CORES)], axis=0)
    return out.astype(np.float32)


# revision 26
# speedup vs baseline: 1.0001x; 1.0001x over previous
"""Bass/Tile TRN2 kernel for nn_FCMTLSTMFull (CNN + BiLSTM + attention + MoE head).

kernel(**inputs) takes FULL unsharded inputs and returns the FULL (8192, 1)
float32 output.  Batch sharded 8 ways (data parallel), params replicated, one
fused program SPMD on cores 0-7.

v2 design vs the 753us baseline:
 - conv2 runs TWO fp8 DoubleRow passes per (sample, mo) instead of three:
   pass0 pairs taps (t0_hi, t2_hi) at rhs pair-stride 2; pass1 pairs
   (t1_hi, t1_lo) at pair-stride 0 (full-precision center tap).  fp8
   ranges are re-centered (h1 x16, conv2 weights x8, folded into b2f/dwT)
   to cut subnormal quantization noise, which keeps overall rel-err at
   the 3-pass baseline level.
 - maxpool drain rebalanced: conv1 drains on ACT; conv2 PSUM first-touch
   is a DVE tensor_tensor max pairing (or ACT copy + DVE pair, knob C2R);
   the rest of the max pyramid runs span-wide on DVE in bf16 (2x mode);
   GPSIMD does the final bias+relu.
 - conv work is emitted piece-wise between LSTM gate groups so PE / ACT /
   DVE stay concurrently busy through the whole LSTM phase; PSUM is two
   shared 2-bank pools (double buffered) covering fp/conv1/gates/head and
   conv2/scores respectively.
"""

import math
import os
from contextlib import ExitStack

import numpy as np
import ml_dtypes

import concourse.bass as bass
import concourse.mybir as mybir
import concourse.tile as tile
from concourse import bacc
from concourse.bass import ds

F32 = mybir.dt.float32
BF16 = mybir.dt.bfloat16
FP8 = mybir.dt.float8e4
I32 = mybir.dt.int32
AF = mybir.ActivationFunctionType
ALU = mybir.AluOpType
AX = mybir.AxisListType
PM = mybir.MatmulPerfMode

BF = ml_dtypes.bfloat16
F8 = ml_dtypes.float8_e4m3fn

N_CORES = 8
B_FULL = 8192
IN = 184
H = 128
SEQ = 4
FPS = 46
EPS = 1e-5

LP = IN + 2          # padded per-sample length (186)
SPAN = 16            # samples per conv span
WSP = SPAN * LP      # span window elems (2976)

S_H = 16.0           # h1 fp8 scale (folded: c1w,b1f x16; praw scale below)
S_W = 8.0            # conv2 weight fp8 scale (w2 x8 -> praw = 128*y2)
S_Y = S_H * S_W      # praw scale; b2f x S_Y, dwT / S_Y

C2R = os.environ.get("C2R", "DDDDDDDD")   # conv2 drain route per tile (len 8)
C1R = os.environ.get("C1R", "AAAA")       # conv1 drain route per quad
TICKN = int(os.environ.get("TICKN", "3"))  # conv pieces per LSTM gate group
QC2 = int(os.environ.get("QC2", "0"))      # conv2 psum at quad (4-sample) grain
TICKA = int(os.environ.get("TICKA", "1"))  # conv pieces per attention group


def build_program(bc):
    nc = bacc.Bacc(None, debug=False)

    spans = bc // SPAN
    ncol = 512
    nh = bc // ncol

    di = {}

    def inp(name, shape, dt):
        di[name] = nc.dram_tensor(name, list(shape), dt, kind="ExternalInput")
        return di[name]

    # activations
    inp("xph", (1, bc * LP + 68), FP8)
    inp("xpl", (1, bc * LP + 68), FP8)
    inp("xTa", (128, bc), BF16)
    inp("xTb", (56, bc), BF16)
    inp("lab", (1, bc), BF16)
    inp("lab4", (4, bc), BF16)
    inp("onesd", (1, bc), BF16)
    # conv weights
    inp("c1w", (10, 2, 128), FP8)        # [k10, pair, m]
    inp("b1f", (128, 1), F32)
    inp("w2p", (128, 2, 2, 2, 128), FP8)  # [k, mo, pass, pair, m]
    inp("b2f", (128, 2), F32)
    # temporal (bf16)
    inp("fpTa", (128, IN), BF16)
    inp("fpTb", (56, IN), BF16)
    inp("fpb", (FPS, SEQ), F32)
    inp("wih0T", (47, 2, 512), BF16)     # row 46 = bias
    inp("whh0T", (128, 2, 512), BF16)
    inp("wih1T", (128, 2, 2, 512), BF16)
    inp("whh1T", (128, 2, 512), BF16)
    inp("b1l", (1, 2, 4, 128), BF16)     # lay1 bias rows [1, d, g, m]
    inp("awT", (128, 2, 2, 128), BF16)   # [k, ki, mo, m]
    inp("awb", (1, 2, 128), BF16)
    inp("av", (128, 2), BF16)
    # head (bf16)
    inp("dwT", (128, 2, 2, 128), BF16)   # [k, ki, mo, m]
    inp("e1T", (128, 4, 512), BF16)      # [k, ki, 4*128]
    inp("e1b", (1, 4, 128), BF16)
    inp("e2T", (128, 4, 256), BF16)
    inp("e2b", (1, 2, 128), BF16)
    inp("sw1T", (128, 2, 2, 128), BF16)  # [k, e, ki, m]
    inp("sb1", (1, 2, 128), BF16)
    inp("sw2T", (128, 2), BF16)
    inp("sb2", (1, 2), F32)
    inp("lw1T", (128, 2, 2, 2, 128), BF16)  # [k, e, ki, mo, m]
    inp("lb1", (1, 2, 2, 128), BF16)
    inp("lw2T", (128, 2, 2), BF16)          # [k, e, ki]
    inp("lb2", (1, 2), F32)
    inp("awbP", (128, 2), F32)
    inp("e1bP", (128, 4), F32)
    inp("e2bP", (128, 2), F32)
    inp("sb1P", (128, 2), F32)
    inp("lb1P", (128, 4), F32)
    inp("eb2P", (4, 1), F32)
    inp("iota4", (4, 1), F32)
    inp("ones4", (4, 1), F32)
    out_d = nc.dram_tensor("out", [bc, 1], F32, kind="ExternalOutput")

    with tile.TileContext(nc) as tc:
        with ExitStack() as ctx:
            wp = ctx.enter_context(tc.tile_pool(name="wp", bufs=1))
            pb = ctx.enter_context(tc.tile_pool(name="pb", bufs=1))
            wk = ctx.enter_context(tc.tile_pool(name="wk", bufs=2))
            cv = ctx.enter_context(tc.tile_pool(name="cv", bufs=2))
            yh = ctx.enter_context(tc.tile_pool(name="yh", bufs=2))
            # PSUM: two shared 2-bank pools, double-buffered (8 banks total)
            psA = ctx.enter_context(tc.tile_pool(name="psA", bufs=2,
                                                 space="PSUM"))
            psB = ctx.enter_context(tc.tile_pool(name="psB",
                                                 bufs=(1 if QC2 else 2),
                                                 space="PSUM"))

            xp10_tiles = {}

            def get_xp10(sp):
                if sp not in xp10_tiles:
                    xp10 = cv.tile([10, WSP + 12], FP8, tag="xp10",
                                   name=f"xp10_{sp}")
                    s0 = sp * SPAN
                    nc.sync.dma_start(
                        xp10[0:5, :],
                        bass.AP(di["xph"], s0 * LP, [[1, 5], [1, WSP + 12]]))
                    nc.sync.dma_start(
                        xp10[5:10, :],
                        bass.AP(di["xpl"], s0 * LP, [[1, 5], [1, WSP + 12]]))
                    xp10_tiles[sp] = xp10
                return xp10_tiles[sp]

            get_xp10(0)
            get_xp10(1)

            W = {}
            _early = ["c1w", "b1f", "w2p", "b2f", "fpTa", "fpTb", "fpb",
                      "wih0T", "whh0T", "xTa", "xTb"]
            _late = []

            def _load(name):
                d = di[name]
                t = wp.tile(list(d.shape), d.dtype, name=f"W{name}",
                            tag=f"W{name}")
                nc.sync.dma_start(t[:], d[:])
                W[name] = t

            for name in di:
                if name in ("xph", "xpl", "lab", "lab4", "onesd"):
                    continue
                if name in _early:
                    _load(name)
                else:
                    _late.append(name)
            xTlo = W["xTa"][:]
            xThi = W["xTb"][:]

            ones = pb.tile([1, bc], BF16, name="ones", tag="ones")
            nc.sync.dma_start(ones[:], di["onesd"][:])

            # ------------- conv state -------------
            pool_ = pb.tile([128, 2, bc], BF16, name="pool", tag="pool")

            h1sp = ctx.enter_context(tc.tile_pool(name="h1sp", bufs=2))
            h1s_bufs = [h1sp.tile([128, WSP + 8], FP8, name=f"h1s{i}",
                                  tag="h1s") for i in range(2)]
            for b in h1s_bufs:
                nc.gpsimd.memset(b[:], 0.0)
            NA = C2R.count("A")   # A-route tiles per span (pattern len 8)
            assert len(C2R) == 8 and (NA == 0 or 8 % NA == 0)
            ysp_bufs = [yh.tile([128, max(NA, 1) * 2, 2, IN], BF16,
                                name=f"yspA{i}", tag="yspA") for i in range(2)]
            praw_bufs = [wk.tile([128, SPAN, 2], BF16, tag="praw", bufs=2,
                                 name=f"praw{i}") for i in range(2)]

            def emit_conv1(sp, qi):
                xv = get_xp10(sp)[:]
                h1v = h1s_bufs[sp % 2][:]
                pc1 = psA.tile([128, 4, 256], F32, tag="pa",
                               name=f"c1_{sp}_{qi}")
                for si in range(4):
                    s = qi * 4 + si
                    src = bass.AP(xv.tensor, xv.offset + s * LP,
                                  [xv.ap[0]] + [[2, 2], [1, IN]])
                    nc.tensor.matmul(pc1[:, si, 0:IN], W["c1w"][:], src,
                                     start=True, stop=True,
                                     perf_mode=PM.DoubleRow)
                dst = bass.AP(h1v.tensor, h1v.offset + (qi * 4) * LP + 1,
                              [h1v.ap[0]] + [[LP, 4], [1, IN]])
                route = C1R[qi % len(C1R)]
                if route == "A":
                    nc.scalar.activation(dst, pc1[:, :, 0:IN], AF.Relu,
                                         bias=W["b1f"][:, 0:1], scale=1.0)
                else:
                    nc.vector.tensor_scalar(dst, pc1[:, :, 0:IN],
                                            W["b1f"][:, 0:1], 0.0,
                                            ALU.add, ALU.max)

            def emit_conv2(sp, qi):
                h1v = h1s_bufs[sp % 2][:]
                yspA = ysp_bufs[sp % 2]
                praw_s = praw_bufs[sp % 2]
                if QC2:
                    p2q = psB.tile([128, 4, 2, 256], F32, tag="pb",
                                   name=f"c2q_{sp}_{qi}")
                for ti in range(2):
                    tidx = qi * 2 + ti
                    if QC2:
                        p2 = p2q[:, ds(ti * 2, 2), :, :]
                    else:
                        p2 = psB.tile([128, 2, 2, 256], F32, tag="pb",
                                      name=f"c2_{sp}_{tidx}")
                    for si in range(2):
                        s = tidx * 2 + si
                        base = s * LP
                        srcA = bass.AP(h1v.tensor, h1v.offset + base,
                                       [h1v.ap[0]] + [[2, 2], [1, IN]])
                        srcB = bass.AP(h1v.tensor, h1v.offset + base + 1,
                                       [h1v.ap[0]] + [[0, 2], [1, IN]])
                        for mo in range(2):
                            nc.tensor.matmul(p2[:, si, mo, 0:IN],
                                             W["w2p"][:, mo, 0], srcA,
                                             start=True, stop=False,
                                             perf_mode=PM.DoubleRow)
                            nc.tensor.matmul(p2[:, si, mo, 0:IN],
                                             W["w2p"][:, mo, 1], srcB,
                                             start=False, stop=True,
                                             perf_mode=PM.DoubleRow)
                    if not QC2:
                        route = C2R[tidx % len(C2R)]
                        if route == "A":
                            aslot = C2R[:tidx].count("A")
                            nc.scalar.activation(
                                yspA[:, ds(aslot * 2, 2), :, :],
                                p2[:, :, :, 0:IN], AF.Copy)
                        else:
                            nc.vector.tensor_reduce(
                                praw_s[:, ds(tidx * 2, 2), :],
                                p2[:, :, :, 0:IN], axis=AX.X, op=ALU.max)
                if QC2:
                    nc.vector.tensor_reduce(
                        praw_s[:, ds(qi * 4, 4), :],
                        p2q[:, :, :, 0:IN], axis=AX.X, op=ALU.max)

            def emit_chain(sp):
                praw_s = praw_bufs[sp % 2]
                if NA > 0:
                    yspA = ysp_bufs[sp % 2]
                    na2 = NA * 2
                    t1 = wk.tile([128, na2, 2, 92], BF16, tag="pyr1", bufs=2,
                                 name=f"t1_{sp}")
                    nc.vector.tensor_tensor(t1[:], yspA[:, :, :, 0:92],
                                            yspA[:, :, :, 92:184], ALU.max)
                    t2 = wk.tile([128, na2, 2, 46], BF16, tag="pyr2", bufs=2,
                                 name=f"t2_{sp}")
                    nc.vector.tensor_tensor(t2[:], t1[:, :, :, 0:46],
                                            t1[:, :, :, 46:92], ALU.max)
                    t3 = wk.tile([128, na2, 2, 23], BF16, tag="pyr3", bufs=2,
                                 name=f"t3_{sp}")
                    nc.vector.tensor_tensor(t3[:], t2[:, :, :, 0:23],
                                            t2[:, :, :, 23:46], ALU.max)
                    # scatter the A-tile results into praw (stride 8/NA tiles)
                    pv = praw_s[:]
                    adst = bass.AP(pv.tensor, pv.offset,
                                   [pv.ap[0]] + [[2 * 16 // NA, NA],
                                                 [2, 2], [1, 2]])
                    nc.vector.tensor_reduce(adst, t3[:], axis=AX.X,
                                            op=ALU.max)
                # bias + relu on gpsimd -> pool
                for mo in range(2):
                    nc.gpsimd.tensor_scalar(
                        pool_[:, mo, ds(sp * SPAN, SPAN)], praw_s[:, :, mo],
                        W["b2f"][:, mo:mo + 1], 0.0, ALU.add, ALU.max)

            TOTAL_PIECES = (spans + 1) * 4
            piece_ctr = [0]

            def tick(n=1):
                for _ in range(n):
                    p = piece_ctr[0]
                    if p >= TOTAL_PIECES:
                        return
                    piece_ctr[0] += 1
                    sp, qi = p // 4, p % 4
                    if sp < spans:
                        emit_conv1(sp, qi)
                    if sp >= 1:
                        emit_conv2(sp - 1, qi)
                        if qi == 3:
                            emit_chain(sp - 1)

            # ---------------- fp projection -> xt tiles (47 rows) ---------
            tick(int(os.environ.get("TICK0", "6")))
            xt = [pb.tile([47, bc], BF16, name=f"xt{t}", tag=f"xt{t}")
                  for t in range(SEQ)]
            for t in range(SEQ):
                nc.sync.dma_start(xt[t][46:47, :], di["onesd"][:])
            for t in range(SEQ):
                for n in range(nh):
                    cols = ds(n * ncol, ncol)
                    ps = psA.tile([46, 1024], F32, tag="pa", name=f"fp{t}_{n}")
                    nc.tensor.matmul(ps[:, 0:ncol],
                                     W["fpTa"][:, ds(t * FPS, FPS)],
                                     xTlo[:, cols], start=True, stop=False)
                    nc.tensor.matmul(ps[:, 0:ncol],
                                     W["fpTb"][:, ds(t * FPS, FPS)],
                                     xThi[:, cols], start=False, stop=True)
                    nc.scalar.activation(xt[t][0:46, cols], ps[:, 0:ncol],
                                         AF.Identity,
                                         bias=W["fpb"][:, t:t + 1], scale=1.0)
                    tick(1)

            for name in _late:
                _load(name)

            # ---------------- LSTM ----------------
            h0 = {}
            h1 = {}
            for (lay, hs) in ((0, h0), (1, h1)):
                for t in range(SEQ):
                    for d in range(2):
                        hs[(t, d)] = pb.tile([128, bc], BF16,
                                             name=f"h{lay}_{t}_{d}",
                                             tag=f"h{lay}_{t}_{d}")

            def lstm_dir(lay, hs, d):
                h_prev = None
                c_prev = None
                for step in range(SEQ):
                    t = step if d == 0 else SEQ - 1 - step
                    c_cur = wk.tile([128, bc], BF16, tag="c", bufs=2,
                                    name=f"c{lay}_{d}_{step}")
                    gv = {}
                    for gpair in ((0, 1), (2, 3)):
                        gt = wk.tile([128, 2, bc], BF16, tag=f"gt{gpair[0]}",
                                     bufs=2, name=f"gt{lay}_{d}_{step}_{gpair[0]}")
                        for n in range(nh):
                            cols = ds(n * ncol, ncol)
                            ps = psA.tile([128, 2, 512], F32, tag="pa",
                                          name=f"g{lay}_{d}_{step}_{gpair[0]}_{n}")
                            for gi_, g in enumerate(gpair):
                                if step == 0 and g == 1:
                                    # f-gate unused at step 0; cheap filler
                                    nc.tensor.matmul(
                                        ps[:, gi_, 0:ncol], W["b1l"][:, d, g],
                                        ones[0:1, cols], start=True, stop=True)
                                    continue
                                gs = ds(g * 128, 128)
                                if lay == 0:
                                    nc.tensor.matmul(
                                        ps[:, gi_, 0:ncol],
                                        W["wih0T"][:, d, gs], xt[t][:, cols],
                                        start=True, stop=(step == 0))
                                else:
                                    nc.tensor.matmul(
                                        ps[:, gi_, 0:ncol],
                                        W["wih1T"][:, d, 0, gs],
                                        h0[(t, 0)][:, cols],
                                        start=True, stop=False)
                                    nc.tensor.matmul(
                                        ps[:, gi_, 0:ncol],
                                        W["wih1T"][:, d, 1, gs],
                                        h0[(t, 1)][:, cols],
                                        start=False, stop=False)
                                    nc.tensor.matmul(
                                        ps[:, gi_, 0:ncol],
                                        W["b1l"][:, d, g], ones[0:1, cols],
                                        start=False, stop=(step == 0))
                                if step > 0:
                                    nc.tensor.matmul(
                                        ps[:, gi_, 0:ncol],
                                        W[f"whh{lay}T"][:, d, gs],
                                        h_prev[:, cols],
                                        start=False, stop=True)
                            if gpair == (0, 1):
                                nc.scalar.activation(gt[:, :, cols],
                                                     ps[:, :, 0:ncol],
                                                     AF.Sigmoid)
                            else:
                                nc.scalar.activation(gt[:, 0, cols],
                                                     ps[:, 0, 0:ncol],
                                                     AF.Tanh)
                                nc.scalar.activation(gt[:, 1, cols],
                                                     ps[:, 1, 0:ncol],
                                                     AF.Sigmoid)
                            tick(TICKN)
                        gv[gpair] = gt
                    g_if = gv[(0, 1)]
                    g_go = gv[(2, 3)]
                    if step == 0:
                        nc.vector.tensor_mul(c_cur[:], g_if[:, 0, :],
                                             g_go[:, 0, :])
                    else:
                        ig = wk.tile([128, bc], BF16, tag="ig", bufs=1,
                                     name=f"ig{lay}_{d}_{step}")
                        nc.vector.tensor_mul(ig[:], g_if[:, 0, :], g_go[:, 0, :])
                        nc.vector.tensor_mul(c_cur[:], g_if[:, 1, :], c_prev[:])
                        nc.vector.tensor_add(c_cur[:], c_cur[:], ig[:])
                    tch = wk.tile([128, bc], BF16, tag="tch", bufs=1,
                                  name=f"tc{lay}_{d}_{step}")
                    nc.scalar.activation(tch[:], c_cur[:], AF.Tanh)
                    nc.vector.tensor_mul(hs[(t, d)][:], g_go[:, 1, :], tch[:])
                    h_prev = hs[(t, d)]
                    c_prev = c_cur

            lstm_dir(0, h0, 0)
            lstm_dir(0, h0, 1)
            lstm_dir(1, h1, 0)
            lstm_dir(1, h1, 1)

            # ---------------- attention ----------------
            E = [pb.tile([1, bc], BF16, name=f"E{t}", tag=f"E{t}")
                 for t in range(SEQ)]
            for t in range(SEQ):
                u = wk.tile([128, 2, bc], BF16, tag="u", bufs=1, name=f"u{t}")
                for n in range(nh):
                    cols = ds(n * ncol, ncol)
                    ps = psA.tile([128, 2, 512], F32, tag="pa",
                                  name=f"at{t}_{n}")
                    for mo in range(2):
                        nc.tensor.matmul(ps[:, mo, 0:ncol],
                                         W["awT"][:, 0, mo], h1[(t, 0)][:, cols],
                                         start=True, stop=False)
                        nc.tensor.matmul(ps[:, mo, 0:ncol],
                                         W["awT"][:, 1, mo], h1[(t, 1)][:, cols],
                                         start=False, stop=False)
                        nc.tensor.matmul(ps[:, mo, 0:ncol],
                                         W["awb"][:, mo], ones[0:1, cols],
                                         start=False, stop=True)
                    nc.scalar.activation(u[:, :, cols], ps[:, :, 0:ncol],
                                         AF.Tanh)
                    tick(TICKA)
                for n in range(nh):
                    cols = ds(n * ncol, ncol)
                    ps = psB.tile([1, 1024], F32, tag="pb", name=f"sc{t}_{n}")
                    nc.tensor.matmul(ps[0:1, 0:ncol], W["av"][:, 0:1],
                                     u[:, 0, cols], start=True, stop=False)
                    nc.tensor.matmul(ps[0:1, 0:ncol], W["av"][:, 1:2],
                                     u[:, 1, cols], start=False, stop=True)
                    nc.scalar.activation(E[t][0:1, cols], ps[0:1, 0:ncol],
                                         AF.Exp)
                    tick(TICKA)
            SE = wk.tile([1, bc], BF16, tag="se", bufs=1, name="SE")
            nc.vector.tensor_add(SE[:], E[0][:], E[1][:])
            nc.vector.tensor_add(SE[:], SE[:], E[2][:])
            nc.vector.tensor_add(SE[:], SE[:], E[3][:])
            Rr = pb.tile([1, bc], BF16, name="Rr", tag="Rr")
            with nc.allow_low_precision("softmax weights tolerate bf16"):
                nc.vector.reciprocal(Rr[:], SE[:])
            ctxb = pb.tile([128, 2, bc], BF16, name="ctxb", tag="ctxb")
            for s in range(SEQ):
                As = wk.tile([1, bc], BF16, tag="As", bufs=2, name=f"As{s}")
                nc.vector.tensor_mul(As[:], E[s][:], Rr[:])
                AW = wk.tile([128, bc], BF16, tag="AW", bufs=1, name=f"AW{s}")
                nc.gpsimd.partition_broadcast(AW[:], As[0:1, :], channels=128)
                for p in range(2):
                    if s == 0:
                        nc.vector.tensor_mul(ctxb[:, p, :], h1[(0, p)][:], AW[:])
                    else:
                        cm = wk.tile([128, bc], BF16, tag="cm", bufs=1,
                                     name=f"cm{s}_{p}")
                        nc.vector.tensor_mul(cm[:], h1[(s, p)][:], AW[:])
                        nc.vector.tensor_add(ctxb[:, p, :], ctxb[:, p, :],
                                             cm[:])
                tick(1)

            # ------------- head (fused per column chunk) -------------
            # chunk n only needs pool_ for spans [n*32, n*32+32); chunk 0's
            # spans finished during the LSTM phase, so leftover conv pieces
            # spread through chunk-0 compute and the rest flush before
            # chunk 1.
            zsp = pb.tile([128, 2, bc], BF16, name="zsp", tag="zsp")
            z2 = pb.tile([128, 4, bc], BF16, name="z2", tag="z2")
            enc = pb.tile([128, 2, bc], BF16, name="enc", tag="enc")
            pred = pb.tile([1, bc], F32, name="pred", tag="pred")
            Lf = pb.tile([1, bc], BF16, name="Lf", tag="Lf")
            nc.sync.dma_start(Lf[:], di["lab"][:])
            for n in range(nh):
                if n == 1:
                    tick(TOTAL_PIECES)   # chunk 1 needs every span's pool
                cols = ds(n * ncol, ncol)
                ps = psA.tile([128, 2, 512], F32, tag="pa", name=f"sp_{n}")
                for mo in range(2):
                    for ki in range(2):
                        nc.tensor.matmul(ps[:, mo, 0:ncol],
                                         W["dwT"][:, ki, mo],
                                         pool_[:, ki, cols],
                                         start=(ki == 0), stop=(ki == 1))
                nc.scalar.activation(zsp[:, :, cols], ps[:, :, 0:ncol], AF.Copy)
                tick(4)
                for mop in range(2):
                    ps = psB.tile([128, 2, 512], F32, tag="pb",
                                  name=f"e1_{mop}_{n}")
                    for mi in range(2):
                        mo = mop * 2 + mi
                        ms = ds(mo * 128, 128)
                        nc.tensor.matmul(ps[:, mi, 0:ncol], W["e1T"][:, 0, ms],
                                         zsp[:, 0, cols], start=True, stop=False)
                        nc.tensor.matmul(ps[:, mi, 0:ncol], W["e1T"][:, 1, ms],
                                         zsp[:, 1, cols], start=False, stop=False)
                        nc.tensor.matmul(ps[:, mi, 0:ncol], W["e1T"][:, 2, ms],
                                         ctxb[:, 0, cols], start=False, stop=False)
                        nc.tensor.matmul(ps[:, mi, 0:ncol], W["e1T"][:, 3, ms],
                                         ctxb[:, 1, cols], start=False, stop=False)
                        nc.tensor.matmul(ps[:, mi, 0:ncol], W["e1b"][:, mo],
                                         ones[0:1, cols], start=False, stop=True)
                    nc.scalar.activation(z2[:, ds(mop * 2, 2), cols],
                                         ps[:, :, 0:ncol], AF.Relu)
                    tick(4)
                ps = psB.tile([128, 2, 512], F32, tag="pb", name=f"e2_{n}")
                for mo in range(2):
                    ms = ds(mo * 128, 128)
                    for ki in range(4):
                        nc.tensor.matmul(ps[:, mo, 0:ncol], W["e2T"][:, ki, ms],
                                         z2[:, ki, cols],
                                         start=(ki == 0), stop=False)
                    nc.tensor.matmul(ps[:, mo, 0:ncol], W["e2b"][:, mo],
                                     ones[0:1, cols], start=False, stop=True)
                nc.scalar.activation(enc[:, :, cols], ps[:, :, 0:ncol], AF.Relu)
                tick(4)
                for e in range(4):
                    po = psB.tile([1, 1024], F32, tag="pb", name=f"xo{e}_{n}")
                    if e < 2:
                        ph = psA.tile([128, 2, 512], F32, tag="pa",
                                      name=f"xh{e}_{n}")
                        for ki in range(2):
                            nc.tensor.matmul(ph[:, 0, 0:ncol],
                                             W["sw1T"][:, e, ki],
                                             enc[:, ki, cols],
                                             start=(ki == 0), stop=(ki == 1))
                        hh = wk.tile([128, ncol], BF16, tag="hh", bufs=2,
                                     name=f"hhS{e}_{n}")
                        nc.scalar.activation(hh[:], ph[:, 0, 0:ncol], AF.Relu,
                                             bias=W["sb1P"][:, e:e + 1],
                                             scale=1.0)
                        nc.tensor.matmul(po[0:1, 0:ncol], W["sw2T"][:, e:e + 1],
                                         hh[:], start=True, stop=True)
                        b2ap = W["sb2"][0:1, e:e + 1]
                    else:
                        el = e - 2
                        ph = psA.tile([128, 2, 512], F32, tag="pa",
                                      name=f"xhL{el}_{n}")
                        for mo in range(2):
                            for ki in range(2):
                                nc.tensor.matmul(ph[:, mo, 0:ncol],
                                                 W["lw1T"][:, el, ki, mo],
                                                 enc[:, ki, cols],
                                                 start=(ki == 0), stop=(ki == 1))
                        hh = wk.tile([128, 2, ncol], BF16, tag="hhL", bufs=2,
                                     name=f"hhL{el}_{n}")
                        for mo in range(2):
                            nc.scalar.activation(hh[:, mo, :], ph[:, mo, 0:ncol],
                                                 AF.Relu,
                                                 bias=W["lb1P"][:, 2 * el + mo:
                                                                2 * el + mo + 1],
                                                 scale=1.0)
                        for mo in range(2):
                            nc.tensor.matmul(po[0:1, 0:ncol],
                                             W["lw2T"][:, el, mo:mo + 1],
                                             hh[:, mo, :],
                                             start=(mo == 0), stop=(mo == 1))
                        b2ap = W["lb2"][0:1, el:el + 1]
                    oe = wk.tile([1, ncol], F32, tag="oe", bufs=2,
                                 name=f"oe{e}_{n}")
                    nc.scalar.activation(oe[:], po[0:1, 0:ncol], AF.Identity,
                                         bias=b2ap, scale=1.0)
                    mk = wk.tile([1, ncol], BF16, tag="mk", bufs=2,
                                 name=f"mk{e}_{n}")
                    nc.vector.tensor_scalar(mk[:], Lf[0:1, cols], float(e),
                                            None, ALU.is_equal)
                    if e == 0:
                        nc.vector.tensor_mul(pred[0:1, cols], oe[:], mk[:])
                    else:
                        pm = wk.tile([1, ncol], F32, tag="pm", bufs=2,
                                     name=f"pm{e}_{n}")
                        nc.vector.tensor_mul(pm[:], oe[:], mk[:])
                        nc.vector.tensor_add(pred[0:1, cols], pred[0:1, cols],
                                             pm[:])
                    tick(1)
            nc.sync.dma_start(out_d[:].rearrange("b one -> one b"), pred[:])

    nc.compile()
    return nc


_PROG_CACHE = {}


def _get_program(bc):
    if bc not in _PROG_CACHE:
        _PROG_CACHE[bc] = build_program(bc)
    return _PROG_CACHE[bc]


def prep_arrays(inputs, n_cores=N_CORES):
    f32 = np.float32
    gi = {k: np.asarray(v) for k, v in inputs.items()}
    x = gi["x"].astype(f32)
    lab = gi["group_labels"].astype(np.int32)
    B = x.shape[0]
    bc = B // n_cores

    s = f32(1.0 / math.sqrt(1.0 + EPS))
    g1 = gi["bn1_g"] * s
    b1f = (gi["conv1_b"] + gi["bn1_b"] / g1) * S_H
    w2 = gi["conv2_w"] * g1[None, :, None] * S_W   # (256,128,3), x8
    g2 = gi["bn2_g"] * s
    b2f = ((gi["conv2_b"] + gi["bn2_b"] / g2) * S_Y).reshape(2, 128).T
    dw = gi["cnn_dw"] * g2[None, :] / S_Y          # (256,256)

    # conv1 weights (x S_H): hi/lo residual split, 10-row layout
    # rows 0-4 = x_hi shifts 0..4, rows 5-9 = x_lo shifts 0..4
    w1 = gi["conv1_w"][:, 0, :].T.astype(f32) * S_H   # (3, 128) [tap, c]
    w1hi = w1.astype(F8).astype(f32)
    w1lo = (w1 - w1hi).astype(F8).astype(f32)
    c1w = np.zeros((10, 2, 128), f32)
    c1w[2:5, 0] = w1lo       # W_lo . X_hi
    c1w[7:10, 0] = w1hi      # W_hi . X_lo
    c1w[0:3, 1] = w1hi       # W_hi . X_hi
    c1w[5:8, 1] = w1lo       # W_lo . X_lo
    c1w = c1w.astype(F8)

    # conv2 DR weights: [k, mo, pass, pair, m]
    # pass0 pair (t0_hi, t2_hi) @ stride 2; pass1 pair (t1_hi, t1_lo) @ str 0
    w2p = np.zeros((128, 2, 2, 2, 128), f32)
    wr = w2.reshape(2, 128, 128, 3)           # [mo, m, k, t]
    for mo in range(2):
        t0 = wr[mo, :, :, 0].T
        t1 = wr[mo, :, :, 1].T
        t2 = wr[mo, :, :, 2].T
        t0h = t0.astype(F8).astype(f32)
        t2h = t2.astype(F8).astype(f32)
        t1h = t1.astype(F8).astype(f32)
        w2p[:, mo, 0, 0] = t0h
        w2p[:, mo, 0, 1] = t2h
        w2p[:, mo, 1, 0] = t1h
        w2p[:, mo, 1, 1] = t1 - t1h
    w2p = w2p.astype(F8)

    fpT = gi["fp_w"].T.astype(f32)
    fpb = np.ascontiguousarray(gi["fp_b"].reshape(SEQ, FPS).T).astype(f32)

    def pack_T(w):
        return np.ascontiguousarray(w.T)

    b0 = np.stack([gi["bih0"][d] + gi["bhh0"][d] for d in range(2)])  # (2,512)
    wih0T = np.zeros((47, 2, 512), f32)
    for d in range(2):
        wih0T[0:46, d] = pack_T(gi["Wih0"][d])
        wih0T[46, d] = b0[d]
    whh0T = np.stack([pack_T(gi["Whh0"][d]) for d in range(2)], axis=1)
    wih1 = np.stack([pack_T(gi["Wih1"][d]) for d in range(2)], axis=1)  # (256,2,512)
    wih1T = np.ascontiguousarray(
        wih1.reshape(2, 128, 2, 512).transpose(1, 2, 0, 3))
    whh1T = np.stack([pack_T(gi["Whh1"][d]) for d in range(2)], axis=1)
    b1l = np.stack([gi["bih1"][d] + gi["bhh1"][d] for d in range(2)])
    b1l = np.ascontiguousarray(b1l.reshape(1, 2, 4, 128))

    awT = np.ascontiguousarray(
        gi["attW_w"].T.reshape(2, 128, 2, 128).transpose(1, 0, 2, 3))
    awb = np.ascontiguousarray(gi["attW_b"].reshape(1, 2, 128))
    av = np.ascontiguousarray(gi["attv"].reshape(2, 128).T)

    dwT = np.ascontiguousarray(
        dw.reshape(2, 128, 2, 128).transpose(3, 2, 0, 1))

    e1b_fold = gi["enc1_b"] + gi["enc1_w"][:, :256] @ gi["cnn_db"]
    es1 = np.ones(512, f32) * s * gi["bne1_g"]
    e1w = gi["enc1_w"] * es1[:, None]
    e1b = (e1b_fold * es1 + gi["bne1_b"]).reshape(1, 4, 128)
    e1T = np.ascontiguousarray(
        e1w.T.reshape(4, 128, 512).transpose(1, 0, 2))
    es2 = np.ones(256, f32) * s * gi["bne2_g"]
    e2w = gi["enc2_w"] * es2[:, None]
    e2b = (gi["enc2_b"] * es2 + gi["bne2_b"]).reshape(1, 2, 128)
    e2T = np.ascontiguousarray(
        e2w.T.reshape(4, 128, 256).transpose(1, 0, 2))

    sw1T = np.ascontiguousarray(
        np.stack([gi["decS_w1"][e].T.reshape(2, 128, 128) for e in range(2)],
                 axis=0).transpose(2, 0, 1, 3))
    sb1 = np.ascontiguousarray(
        np.stack([gi["decS_b1"][e] for e in range(2)], axis=0).reshape(1, 2, 128))
    sw2T = np.ascontiguousarray(
        np.stack([gi["decS_w2"][e, 0] for e in range(2)], axis=1))
    sb2 = gi["decS_b2"].reshape(1, 2).astype(f32)
    lw1T = np.ascontiguousarray(
        np.stack([gi["decL_w1"][e].T.reshape(2, 128, 2, 128) for e in range(2)],
                 axis=0).transpose(2, 0, 1, 3, 4))
    lb1 = np.ascontiguousarray(
        np.stack([gi["decL_b1"][e].reshape(2, 128) for e in range(2)],
                 axis=0).reshape(1, 2, 2, 128))
    lw2T = np.ascontiguousarray(
        np.stack([gi["decL_w2"][e, 0].reshape(2, 128) for e in range(2)],
                 axis=0).transpose(2, 0, 1))
    lb2 = gi["decL_b2"].reshape(1, 2).astype(f32)

    awbP = np.ascontiguousarray(awb[0].T).astype(f32)          # (128,2)
    e1bP = np.ascontiguousarray(e1b[0].T).astype(f32)          # (128,4)
    e2bP = np.ascontiguousarray(e2b[0].T).astype(f32)          # (128,2)
    sb1P = np.ascontiguousarray(sb1[0].T).astype(f32)          # (128,2)
    lb1P = np.ascontiguousarray(
        lb1[0].reshape(4, 128).T).astype(f32)                  # (128,4)
    eb2P = np.array([[sb2[0, 0]], [sb2[0, 1]],
                     [lb2[0, 0]], [lb2[0, 1]]], f32)           # (4,1)
    iota4 = np.arange(4, dtype=f32).reshape(4, 1)
    ones4 = np.ones((4, 1), f32)

    shared = dict(
        awbP=awbP, e1bP=e1bP, e2bP=e2bP, sb1P=sb1P, lb1P=lb1P,
        eb2P=eb2P, iota4=iota4, ones4=ones4,
        c1w=c1w, b1f=b1f.reshape(128, 1).astype(f32),
        w2p=w2p, b2f=np.ascontiguousarray(b2f).astype(f32),
        fpTa=np.ascontiguousarray(fpT[0:128]).astype(BF),
        fpTb=np.ascontiguousarray(fpT[128:IN]).astype(BF),
        fpb=fpb,
        wih0T=wih0T.astype(BF), whh0T=whh0T.astype(BF),
        wih1T=wih1T.astype(BF), whh1T=whh1T.astype(BF),
        b1l=b1l.astype(BF),
        awT=awT.astype(BF), awb=awb.astype(BF), av=av.astype(BF),
        dwT=dwT.astype(BF),
        e1T=e1T.astype(BF), e1b=e1b.astype(BF),
        e2T=e2T.astype(BF), e2b=e2b.astype(BF),
        sw1T=sw1T.astype(BF), sb1=sb1.astype(BF), sw2T=sw2T.astype(BF),
        sb2=sb2,
        lw1T=lw1T.astype(BF), lb1=lb1.astype(BF), lw2T=lw2T.astype(BF),
        lb2=lb2,
    )
    per_core = []
    xhi = x.astype(F8).astype(f32)
    xlo = (x - xhi).astype(F8).astype(f32)
    for c in range(n_cores):
        sl = slice(c * bc, (c + 1) * bc)

        def flat_pad(xa):
            flat = np.zeros((1, bc * LP + 68), f32)
            flat[0, 2:2 + bc * LP] = np.pad(xa[sl], ((0, 0), (1, 1))).reshape(-1)
            return flat.astype(F8)

        per_core.append(dict(
            xph=flat_pad(xhi),
            xpl=flat_pad(xlo),
            xTa=np.ascontiguousarray(x[sl].T[0:128]).astype(BF),
            xTb=np.ascontiguousarray(x[sl].T[128:IN]).astype(BF),
            onesd=np.ones((1, bc), BF),
            lab=np.ascontiguousarray(lab[sl].reshape(1, bc).astype(BF)),
            lab4=np.ascontiguousarray(
                np.broadcast_to(lab[sl].reshape(1, bc), (4, bc)).astype(BF)),
        ))
    return shared, per_core, bc


def kernel(**inputs) -> np.ndarray:
    from concourse.bass_utils import run_bass_kernel_spmd

    shared, per_core, bc = prep_arrays(inputs)
    nc = _get_program(bc)
    in_maps = [dict(shared, **pc) for pc in per_core]
    res = run_bass_kernel_spmd(nc, in_maps, core_ids=list(range(N_CORES)))
    out = np.concatenate([res.results[c]["out"] for c in range(N_CORES)], axis=0)
    return out.astype(np.float32)
